# revision 46
# baseline (speedup 1.0000x reference)
"""GRU decoder Bass kernel for Trainium2, data-parallel over batch on 8 cores.

Math refactoring (exactly equivalent to the reference up to fp assoc.):
  context = hidden[0] is constant across steps, and x_{t} = fc_out_{t-1} is
  linear in [h_t, context].  Folding fc into the input projection:
    gi_t = h_t @ M1.T + CONST          (M1 = W_ih @ fc_W[:, :H], t >= 1)
    gh_t = h_t @ W_hh.T + b_hh
  r/z gates add gi+gh, so P_r = M1_r + W_hh_r, P_z = M1_z + W_hh_z fuse into
  one [4096, 1024] weight:  G_t = h_t @ [P_r | P_z | M1_n | W_hh_n].T + C
  fc_out_t = h_{t+1} @ F1.T + CF with F1 = fc_W[:, :H].  Step-0 pre-gates are
  computed on the host.

Layout: per core B=64 batch rows, "split layout": every [64, 1024] tensor is
[128 partitions, 512]: partitions 0-63 = batch rows (hidden dims 0-511),
partitions 64-127 = same batch rows (hidden dims 512-1023).  Matmuls use M=64
stationaries (hT blocks) with automatic 2x PE column tiling (g=0 writes PSUM
partitions 0-63, g=1 writes 64-127, concurrent) — the PE array is fully used.

Scheduling (the point of this file; 1.87x over the naive-ordered version):
  - Gate banks are computed bank-major in order (h_n, r, i_n, z) so the
    sigmoid/tanh chain overlaps the remaining gate matmuls instead of
    stalling the PE for the whole chain at the end of the step.  This keeps
    the PE HAM clock at K=8/8 (the naive order re-throttled to 1.2 GHz every
    step).
  - pre_z is computed as two [128,256] half-banks so Sig(z) starts before
    the second half's matmuls finish.
  - The chain (Sig_r, t1, t2, Tanh, d, Sig_z, e, h') is column-chunked
    (2 x 256) so the first PE transpose starts as soon as half of h is
    updated.
  - The next step's PSUM bias-init matmuls are emitted at the tail to keep
    the PE busy through the chain tail.
  - h state and chain tail run in fp16 (DVE 2x, fp16 transposes at
    1 cycle/row); pre-activation sums stay fp32 in PSUM (bias constants
    enter PSUM via an fp16 hi+lo identity matmul).
  - Transposes are 4 x [128,128] into two half-banks, drained by two wide
    [128,256] casts that gate the fc matmul pairs.
  - Input DMAs: small step-0 tensors first, the 8 MB W4 split per K-chunk so
    compute starts ~35 us earlier.
"""
import os
import numpy as np

H = 1024
OUT = 768
BATCH = 512
NCORES = 8
B = BATCH // NCORES  # 64

# hT column block b (64 cols each) holds K-chunk SIGMA[b]; transpose c
# produces blocks 2c (chunk c) and 2c+1 (chunk c+4).
SIGMA = (0, 4, 1, 5, 2, 6, 3, 7)

_BUILD_CACHE = {}


def _build(T: int):
    from contextlib import ExitStack
    from concourse import tile, mybir, bacc

    F16 = mybir.dt.float16
    F32 = mybir.dt.float32
    Sig = mybir.ActivationFunctionType.Sigmoid
    Tanh = mybir.ActivationFunctionType.Tanh
    MUL = mybir.AluOpType.mult
    ADD = mybir.AluOpType.add
    SUB = mybir.AluOpType.subtract

    nc = bacc.Bacc("TRN2", target_bir_lowering=False, debug=False,
                   num_devices=NCORES)

    dram = {}
    def din(name, shape, dt):
        dram[name] = nc.dram_tensor(name, list(shape), dt, kind="ExternalInput").ap()
        return dram[name]

    w4_d = din("W4", [128, 8 * 4096], F16)
    f1_d = din("F1", [128, 8 * 768], F16)
    ci_d = din("CINIT", [128, 2560], F16)
    id2_d = din("IDENT2", [128, 64], F16)
    idt_d = din("IDENTT", [128, 128], F16)
    h0s_d = din("H0S", [128, 512], F16)
    g0_d = din("G0", [128, 4 * 512], F32)
    cf_d = din("CF", [128, 384], F32)
    out_d = nc.dram_tensor("OUT", [T * 128, 384], F32, kind="ExternalOutput").ap()

    with tile.TileContext(nc) as tc:
        with ExitStack() as ctx:
            wpool = ctx.enter_context(tc.tile_pool(name="weights", bufs=1))
            state = ctx.enter_context(tc.tile_pool(name="state", bufs=1))
            tmp = ctx.enter_context(tc.tile_pool(name="tmp", bufs=2))
            gps = ctx.enter_context(tc.tile_pool(name="gpsum", bufs=1, space="PSUM"))
            fps = ctx.enter_context(tc.tile_pool(name="fpsum", bufs=1, space="PSUM"))
            tps = ctx.enter_context(tc.tile_pool(name="tpsum", bufs=2, space="PSUM"))

            w4 = wpool.tile([128, 8 * 4096], F16, name="w4")
            f1 = wpool.tile([128, 8 * 768], F16, name="f1")
            ci = wpool.tile([128, 2560], F16, name="ci")
            id2 = wpool.tile([128, 64], F16, name="id2")
            idt = wpool.tile([128, 128], F16, name="idt")
            g0 = wpool.tile([128, 4 * 512], F32, name="g0")
            cf = wpool.tile([128, 384], F32, name="cf")
            h = state.tile([128, 512], F16, name="h")
            hT = state.tile([128, 512], F16, name="hT")

            # Small step-0 tensors first so compute starts while the big W4
            # load streams in behind (W4 split per K-chunk so step-1 G matmuls
            # can begin as chunks land).
            for t_sb, t_d in ((g0, g0_d), (h, h0s_d), (idt, idt_d),
                              (id2, id2_d), (cf, cf_d), (f1, f1_d),
                              (ci, ci_d)):
                nc.sync.dma_start(t_sb[:], t_d[:])
            for k in SIGMA:
                nc.sync.dma_start(w4[:, k * 4096:(k + 1) * 4096],
                                  w4_d[:, k * 4096:(k + 1) * 4096])

            # Gate PSUM banks: pre_r, i_n, h_n full banks; pre_z as two
            # [128,256] half-banks so Sig(z) starts before the second half's
            # matmuls finish (shortens the per-step critical tail).
            gb = {j: gps.tile([128, 512], F32, name=f"gb{j}", tag=f"gb{j}")
                  for j in (0, 2, 3)}
            gz = [gps.tile([128, 256], F32, name=f"gz{a}", tag=f"gz{a}")
                  for a in range(2)]

            def emit_init_rows(bank, rows, cols, n):
                # fp16 constant init of a gate bank via a K=64 identity
                # matmul in row-group `rows` (0 or 64).  Two banks in
                # different row groups run CONCURRENTLY on the PE (row
                # tiling composes with the g0/g1 column pairs).
                for g in range(2):
                    nc.tensor.matmul(
                        bank[64 * g:64 * (g + 1), :],
                        id2[rows:rows + 64, :],
                        ci[rows:rows + 64, cols + g * n: cols + (g + 1) * n],
                        start=True, stop=False)

            def emit_G_bank(j):
                for b in range(8):
                    k = SIGMA[b]
                    lhsT = hT[:, b * 64:(b + 1) * 64]
                    for g in range(2):
                        cc = j * 2 + g
                        nc.tensor.matmul(
                            gb[j][64 * g:64 * (g + 1), :], lhsT,
                            w4[:, k * 4096 + cc * 512: k * 4096 + (cc + 1) * 512],
                            start=False, stop=(b == 7))

            def emit_G_z(a):
                for b in range(8):
                    k = SIGMA[b]
                    lhsT = hT[:, b * 64:(b + 1) * 64]
                    for g in range(2):
                        cc = 2 + g
                        nc.tensor.matmul(
                            gz[a][64 * g:64 * (g + 1), :], lhsT,
                            w4[:, k * 4096 + cc * 512 + a * 256:
                               k * 4096 + cc * 512 + (a + 1) * 256],
                            start=False, stop=(b == 7))

            def emit_transposes_fc(t):
                # h -> hT via 4 [128,128] PE transposes (2 per half-bank),
                # drained by 2 wide casts; fc matmul pairs follow each cast.
                fcp = fps.tile([128, 384], F32, name=f"fcp{t}", tag="fcp")
                for half in range(2):
                    trp = tps.tile([128, 256], F16, name=f"trp{t}_{half}", tag="trp")
                    for c2 in range(2):
                        c = half * 2 + c2
                        nc.tensor.transpose(
                            trp[:, c2 * 128:(c2 + 1) * 128],
                            h[0:128, c * 128:(c + 1) * 128], idt[:, :])
                    nc.vector.tensor_copy(
                        hT[:, half * 256:(half + 1) * 256], trp[:, :])
                for b in range(8):
                    k = SIGMA[b]
                    lhsT = hT[:, b * 64:(b + 1) * 64]
                    for g in range(2):
                        nc.tensor.matmul(
                            fcp[64 * g:64 * (g + 1), :], lhsT,
                            f1[:, k * 768 + g * 384: k * 768 + g * 384 + 384],
                            start=(b == 0), stop=(b == 7))
                st = tmp.tile([128, 384], F32, name=f"st{t}", tag="st")
                nc.vector.tensor_add(st[:], fcp[:], cf[:])
                nc.sync.dma_start(out_d[t * 128:(t + 1) * 128, :], st[:])

            def emit_chain_head(t, pr, phn):
                # r = Sig(pre_r); t1 = r * phn  (column-chunked to pipeline)
                r = tmp.tile([128, 512], F32, name=f"r{t}", tag="r")
                t1 = tmp.tile([128, 512], F32, name=f"t1{t}", tag="t1")
                for a in range(2):
                    s = slice(a * 256, (a + 1) * 256)
                    nc.scalar.activation(r[:, s], pr(a), Sig)
                    nc.vector.tensor_tensor(t1[:, s], r[:, s], phn(a), MUL)
                return t1

            def emit_chain_mid(t, t1, pin):
                # t2 = t1 + i_n; n = Tanh(t2)  (column-chunked to pipeline)
                t2 = tmp.tile([128, 512], F32, name=f"t2{t}", tag="t2")
                n = tmp.tile([128, 512], F16, name=f"n{t}", tag="n")
                for a in range(2):
                    s = slice(a * 256, (a + 1) * 256)
                    nc.vector.tensor_tensor(t2[:, s], t1[:, s], pin(a), ADD)
                    nc.scalar.activation(n[:, s], t2[:, s], Tanh)
                return n

            def emit_chain_tail(t, n, pz):
                # d = h - n; per 256-col chunk: z=Sig(pre_z), e=z*d, h'=n+e
                z = tmp.tile([128, 512], F16, name=f"z{t}", tag="z")
                d = tmp.tile([128, 512], F16, name=f"d{t}", tag="d")
                e = tmp.tile([128, 512], F16, name=f"e{t}", tag="e")
                for a in range(2):
                    s = slice(a * 256, (a + 1) * 256)
                    nc.vector.tensor_tensor(d[:, s], h[:, s], n[:, s], SUB)
                for a in range(2):
                    s = slice(a * 256, (a + 1) * 256)
                    nc.scalar.activation(z[:, s], pz(a), Sig)
                    nc.vector.tensor_tensor(e[:, s], z[:, s], d[:, s], MUL)
                    nc.vector.tensor_tensor(h[:, s], n[:, s], e[:, s], ADD)

            def emit_inits():
                # (h_n, r) share one 4-MM row+col-tiled span; i_n alone;
                # (zA, zB) share one span
                emit_init_rows(gb[3], 0, 0, 512)
                emit_init_rows(gb[0], 64, 0, 512)
                emit_init_rows(gb[2], 0, 1024, 512)
                emit_init_rows(gz[0], 0, 2048, 256)
                emit_init_rows(gz[1], 64, 2048, 256)

            # ---- step 0: gates come from host-computed G0 ----
            t1_0 = emit_chain_head(0,
                                   lambda a: g0[:, a * 256:(a + 1) * 256],
                                   lambda a: g0[:, 1536 + a * 256: 1536 + (a + 1) * 256])
            n_0 = emit_chain_mid(0, t1_0,
                                 lambda a: g0[:, 1024 + a * 256: 1024 + (a + 1) * 256])
            emit_chain_tail(0, n_0,
                            lambda a: g0[:, 512 + a * 256: 512 + (a + 1) * 256])
            # Step 0: transposes+fc first — the bias inits wait on the CINIT
            # DMA (last small input) and would serialize the prologue if they
            # sat ahead in the PE queue; their only deadline is step 1's G.
            emit_transposes_fc(0)
            if T > 1:
                emit_inits()

            for t in range(1, T):
                emit_G_bank(3)   # h_n
                emit_G_bank(0)   # r
                t1_t = emit_chain_head(t,
                                       lambda a: gb[0][:, a * 256:(a + 1) * 256],
                                       lambda a: gb[3][:, a * 256:(a + 1) * 256])
                emit_G_bank(2)   # i_n
                n_t = emit_chain_mid(t, t1_t,
                                     lambda a: gb[2][:, a * 256:(a + 1) * 256])
                emit_G_z(0)      # z first half  (tail starts on its result)
                emit_G_z(1)      # z second half
                emit_chain_tail(t, n_t, lambda a: gz[a][:, :])
                if t + 1 < T:
                    emit_inits()
                emit_transposes_fc(t)

    nc.compile()
    return nc


def _hi_lo(x):
    hi = x.astype(np.float16)
    lo = (x - hi.astype(np.float32)).astype(np.float16)
    return hi, lo


def _split_cols(x):
    """[B, 1024] -> [128, 512] split layout (dims 0-511 on parts 0-63)."""
    return np.concatenate([x[:, :512], x[:, 512:]], axis=0)


def kernel(src, hidden, W_ih, W_hh, b_ih, b_hh, fc_W, fc_b, output_len):
    from concourse import bass_utils

    T = int(output_len)
    src = np.asarray(src, np.float32)
    hidden = np.asarray(hidden, np.float32)
    W_ih = np.asarray(W_ih, np.float32)
    W_hh = np.asarray(W_hh, np.float32)
    b_ih = np.asarray(b_ih, np.float32)
    b_hh = np.asarray(b_hh, np.float32)
    fc_W = np.asarray(fc_W, np.float32)
    fc_b = np.asarray(fc_b, np.float32)

    ctx = hidden[0]          # [B, H]
    h0 = hidden[0]
    x0 = src[0]              # [B, OUT]

    # ---- host weight folding (fp32) ----
    M1 = W_ih @ fc_W[:, :H]          # [3H, H]
    M2 = W_ih @ fc_W[:, H:]          # [3H, H]
    P_r = M1[0:H] + W_hh[0:H]
    P_z = M1[H:2 * H] + W_hh[H:2 * H]
    Wbig4 = np.concatenate([P_r, P_z, M1[2 * H:], W_hh[2 * H:]], axis=0)  # [4096, H]
    F1 = fc_W[:, :H]                 # [OUT, H]

    CONST = ctx @ M2.T + (fc_b @ W_ih.T + b_ih)     # [B, 3H]
    c_r = CONST[:, 0:H] + b_hh[0:H]
    c_z = CONST[:, H:2 * H] + b_hh[H:2 * H]
    c_in = CONST[:, 2 * H:]
    c_hn = np.broadcast_to(b_hh[2 * H:], (BATCH, H)).astype(np.float32)
    CALL = np.stack([c_r, c_z, c_in, c_hn], axis=1)  # [B, 4, H]

    CF = ctx @ fc_W[:, H:].T + fc_b                  # [B, OUT]

    gi0 = x0 @ W_ih.T + b_ih
    gh0 = h0 @ W_hh.T + b_hh
    G0_parts = np.stack([gi0[:, :H] + gh0[:, :H],
                         gi0[:, H:2 * H] + gh0[:, H:2 * H],
                         gi0[:, 2 * H:],
                         gh0[:, 2 * H:]], axis=1)    # [B, 4, H]

    # ---- shared (replicated) tensors ----
    # W4 sbuf layout: [p, k*4096 + cc*512 + c] = Wbig4[1024*j + 512*g + c, 128k+p]
    W4T = Wbig4.T.reshape(8, 128, 8, 512)            # [k, p, cc, c]
    W4 = np.ascontiguousarray(W4T.transpose(1, 0, 2, 3)).reshape(128, 8 * 4096)
    W4 = W4.astype(np.float16)
    # F1 sbuf: [p, k*768 + g*384 + c] = F1[384g+c, 128k+p]
    F1T = F1.T.reshape(8, 128, 2, 384)               # [k, p, g, c]
    F1s = np.ascontiguousarray(F1T.transpose(1, 0, 2, 3)).reshape(128, 8 * 768)
    F1s = F1s.astype(np.float16)
    ID2 = np.concatenate([np.eye(64), np.eye(64)], axis=0).astype(np.float16)
    IDT = np.eye(128).astype(np.float16)

    key = T
    if key not in _BUILD_CACHE:
        _BUILD_CACHE[key] = _build(T)
    nc = _BUILD_CACHE[key]

    in_maps = []
    for c in range(NCORES):
        sl = slice(c * B, (c + 1) * B)
        # CINIT (fp16 constants, row-group packed): rows 0-63 carry h_n, i_n
        # and zA constants; rows 64-127 carry r and zB, so paired bank inits
        # run as concurrent PE row tiles.
        call16 = CALL[sl].astype(np.float16)         # [64, 4, 1024]

        def sc16(j):                                 # [64, 1024] g-major split
            x = call16[:, j, :]
            return np.concatenate([x[:, :512], x[:, 512:]], axis=1)

        ci = np.zeros((128, 2560), np.float16)
        ci[0:64, 0:1024] = sc16(3)                   # h_n  (rows 0-63)
        ci[64:128, 0:1024] = sc16(0)                 # r    (rows 64-127)
        ci[0:64, 1024:2048] = sc16(2)                # i_n
        zz = call16[:, 1, :]                         # pre_z constants
        ci[0:64, 2048:2304] = zz[:, 0:256]           # zA: g0 cols 0-255
        ci[0:64, 2304:2560] = zz[:, 512:768]         # zA: g1
        ci[64:128, 2048:2304] = zz[:, 256:512]       # zB: g0
        ci[64:128, 2304:2560] = zz[:, 768:1024]      # zB: g1

        h0_c = h0[sl]
        H0S = _split_cols(h0_c)                      # [128, 512]

        g0_c = G0_parts[sl]                          # [B, 4, H]
        G0s = np.concatenate([g0_c[:, :, :512], g0_c[:, :, 512:]],
                             axis=0)                 # [128, 4, 512]
        G0s = np.ascontiguousarray(G0s).reshape(128, 4 * 512)

        cf_c = CF[sl].reshape(B, 2, 384)             # [b, g, c]
        CFs = np.ascontiguousarray(cf_c.transpose(1, 0, 2)).reshape(128, 384)

        in_maps.append({
            "W4": W4, "F1": F1s,
            "CINIT": np.ascontiguousarray(ci).astype(np.float16),
            "IDENT2": ID2, "IDENTT": IDT,
            "H0S": np.ascontiguousarray(H0S).astype(np.float16),
            "G0": G0s.astype(np.float32),
            "CF": CFs.astype(np.float32),
        })

    trace = bool(os.environ.get("GRU_TRACE"))
    res = bass_utils.run_bass_kernel_spmd(
        nc, in_maps, core_ids=list(range(NCORES)), trace=trace)
    if trace:
        kernel.last_exec_time_ns = res.exec_time_ns
        kernel.last_results = res

    outs = []
    for c in range(NCORES):
        o = res.results[c]["OUT"]                    # [T*128, 384]
        o = o.reshape(T, 2, B, 384).transpose(0, 2, 1, 3).reshape(T, B, OUT)
        outs.append(o)
    return np.concatenate(outs, axis=1)              # [T, BATCH, OUT]


# revision 47
# speedup vs baseline: 1.0014x; 1.0014x over previous
"""GRU decoder Bass kernel for Trainium2, data-parallel over batch on 8 cores.

Math refactoring (exactly equivalent to the reference up to fp assoc.):
  context = hidden[0] is constant across steps, and x_{t} = fc_out_{t-1} is
  linear in [h_t, context].  Folding fc into the input projection:
    gi_t = h_t @ M1.T + CONST          (M1 = W_ih @ fc_W[:, :H], t >= 1)
    gh_t = h_t @ W_hh.T + b_hh
  r/z gates add gi+gh, so P_r = M1_r + W_hh_r, P_z = M1_z + W_hh_z fuse into
  one [4096, 1024] weight:  G_t = h_t @ [P_r | P_z | M1_n | W_hh_n].T + C
  fc_out_t = h_{t+1} @ F1.T + CF with F1 = fc_W[:, :H].  Step-0 pre-gates are
  computed on the host.

Layout: per core B=64 batch rows, "split layout": every [64, 1024] tensor is
[128 partitions, 512]: partitions 0-63 = batch rows (hidden dims 0-511),
partitions 64-127 = same batch rows (hidden dims 512-1023).  Matmuls use M=64
stationaries (hT blocks) with automatic 2x PE column tiling (g=0 writes PSUM
partitions 0-63, g=1 writes 64-127, concurrent) — the PE array is fully used.

Scheduling (the point of this file; 1.87x over the naive-ordered version):
  - Gate banks are computed bank-major in order (h_n, r, i_n, z) so the
    sigmoid/tanh chain overlaps the remaining gate matmuls instead of
    stalling the PE for the whole chain at the end of the step.  This keeps
    the PE HAM clock at K=8/8 (the naive order re-throttled to 1.2 GHz every
    step).
  - pre_z is computed as two [128,256] half-banks so Sig(z) starts before
    the second half's matmuls finish.
  - The chain (Sig_r, t1, t2, Tanh, d, Sig_z, e, h') is column-chunked
    (2 x 256) so the first PE transpose starts as soon as half of h is
    updated.
  - The next step's PSUM bias-init matmuls are emitted at the tail to keep
    the PE busy through the chain tail.
  - h state and chain tail run in fp16 (DVE 2x, fp16 transposes at
    1 cycle/row); pre-activation sums stay fp32 in PSUM (bias constants
    enter PSUM via an fp16 hi+lo identity matmul).
  - Transposes are 4 x [128,128] into two half-banks, drained by two wide
    [128,256] casts that gate the fc matmul pairs.
  - Input DMAs: small step-0 tensors first, the 8 MB W4 split per K-chunk so
    compute starts ~35 us earlier.
"""
import os
import numpy as np

H = 1024
OUT = 768
BATCH = 512
NCORES = 8
B = BATCH // NCORES  # 64

# hT column block b (64 cols each) holds K-chunk SIGMA[b]; transpose c
# produces blocks 2c (chunk c) and 2c+1 (chunk c+4).
SIGMA = (0, 4, 1, 5, 2, 6, 3, 7)

_BUILD_CACHE = {}


def _build(T: int):
    from contextlib import ExitStack
    from concourse import tile, mybir, bacc

    F16 = mybir.dt.float16
    F32 = mybir.dt.float32
    Sig = mybir.ActivationFunctionType.Sigmoid
    Tanh = mybir.ActivationFunctionType.Tanh
    MUL = mybir.AluOpType.mult
    ADD = mybir.AluOpType.add
    SUB = mybir.AluOpType.subtract

    nc = bacc.Bacc("TRN2", target_bir_lowering=False, debug=False,
                   num_devices=NCORES)

    dram = {}
    def din(name, shape, dt):
        dram[name] = nc.dram_tensor(name, list(shape), dt, kind="ExternalInput").ap()
        return dram[name]

    w4_d = din("W4", [128, 8 * 4096], F16)
    f1_d = din("F1", [128, 8 * 768], F16)
    ci_d = din("CINIT", [128, 8 * 512], F16)
    id2_d = din("IDENT2", [128, 64], F16)
    idt_d = din("IDENTT", [128, 128], F16)
    h0s_d = din("H0S", [128, 512], F16)
    g0_d = din("G0", [128, 4 * 512], F32)
    cf_d = din("CF", [128, 384], F32)
    out_d = nc.dram_tensor("OUT", [T * 128, 384], F32, kind="ExternalOutput").ap()

    with tile.TileContext(nc) as tc:
        with ExitStack() as ctx:
            wpool = ctx.enter_context(tc.tile_pool(name="weights", bufs=1))
            state = ctx.enter_context(tc.tile_pool(name="state", bufs=1))
            tmp = ctx.enter_context(tc.tile_pool(name="tmp", bufs=2))
            gps = ctx.enter_context(tc.tile_pool(name="gpsum", bufs=1, space="PSUM"))
            fps = ctx.enter_context(tc.tile_pool(name="fpsum", bufs=1, space="PSUM"))
            tps = ctx.enter_context(tc.tile_pool(name="tpsum", bufs=2, space="PSUM"))

            w4 = wpool.tile([128, 8 * 4096], F16, name="w4")
            f1 = wpool.tile([128, 8 * 768], F16, name="f1")
            ci = wpool.tile([128, 8 * 512], F16, name="ci")
            id2 = wpool.tile([128, 64], F16, name="id2")
            idt = wpool.tile([128, 128], F16, name="idt")
            g0 = wpool.tile([128, 4 * 512], F32, name="g0")
            cf = wpool.tile([128, 384], F32, name="cf")
            h = state.tile([128, 512], F16, name="h")
            hT = state.tile([128, 512], F16, name="hT")

            # Small step-0 tensors first so compute starts while the big W4
            # load streams in behind (W4 split per K-chunk so step-1 G matmuls
            # can begin as chunks land).
            for t_sb, t_d in ((g0, g0_d), (h, h0s_d), (idt, idt_d),
                              (id2, id2_d), (cf, cf_d), (f1, f1_d),
                              (ci, ci_d)):
                nc.sync.dma_start(t_sb[:], t_d[:])
            for k in SIGMA:
                nc.sync.dma_start(w4[:, k * 4096:(k + 1) * 4096],
                                  w4_d[:, k * 4096:(k + 1) * 4096])

            # Gate PSUM banks: pre_r, i_n, h_n full banks; pre_z as two
            # [128,256] half-banks so Sig(z) starts before the second half's
            # matmuls finish (shortens the per-step critical tail).
            gb = {j: gps.tile([128, 512], F32, name=f"gb{j}", tag=f"gb{j}")
                  for j in (0, 2, 3)}
            gz = [gps.tile([128, 256], F32, name=f"gz{a}", tag=f"gz{a}")
                  for a in range(2)]

            def emit_init_bank(j):
                # bias/constant init of gate bank j (start of its accum group):
                # out[b, c] = ci[b, chunk] + ci[b+64, chunk]  (fp16 hi+lo)
                for g in range(2):
                    cc = j * 2 + g
                    nc.tensor.matmul(
                        gb[j][64 * g:64 * (g + 1), :], id2[:, :],
                        ci[:, cc * 512:(cc + 1) * 512],
                        start=True, stop=False)

            def emit_init_z(a):
                for g in range(2):
                    cc = 2 + g
                    nc.tensor.matmul(
                        gz[a][64 * g:64 * (g + 1), :], id2[:, :],
                        ci[:, cc * 512 + a * 256: cc * 512 + (a + 1) * 256],
                        start=True, stop=False)

            def emit_G_bank(j):
                for b in range(8):
                    k = SIGMA[b]
                    lhsT = hT[:, b * 64:(b + 1) * 64]
                    for g in range(2):
                        cc = j * 2 + g
                        nc.tensor.matmul(
                            gb[j][64 * g:64 * (g + 1), :], lhsT,
                            w4[:, k * 4096 + cc * 512: k * 4096 + (cc + 1) * 512],
                            start=False, stop=(b == 7))

            def emit_G_z(a):
                for b in range(8):
                    k = SIGMA[b]
                    lhsT = hT[:, b * 64:(b + 1) * 64]
                    for g in range(2):
                        cc = 2 + g
                        nc.tensor.matmul(
                            gz[a][64 * g:64 * (g + 1), :], lhsT,
                            w4[:, k * 4096 + cc * 512 + a * 256:
                               k * 4096 + cc * 512 + (a + 1) * 256],
                            start=False, stop=(b == 7))

            def emit_transposes_fc(t):
                # h -> hT via 4 [128,128] PE transposes (2 per half-bank),
                # drained by 2 wide casts; fc matmul pairs follow each cast.
                fcp = fps.tile([128, 384], F32, name=f"fcp{t}", tag="fcp")
                for half in range(2):
                    trp = tps.tile([128, 256], F16, name=f"trp{t}_{half}", tag="trp")
                    for c2 in range(2):
                        c = half * 2 + c2
                        nc.tensor.transpose(
                            trp[:, c2 * 128:(c2 + 1) * 128],
                            h[0:128, c * 128:(c + 1) * 128], idt[:, :])
                    nc.vector.tensor_copy(
                        hT[:, half * 256:(half + 1) * 256], trp[:, :])
                for b in range(8):
                    k = SIGMA[b]
                    lhsT = hT[:, b * 64:(b + 1) * 64]
                    for g in range(2):
                        nc.tensor.matmul(
                            fcp[64 * g:64 * (g + 1), :], lhsT,
                            f1[:, k * 768 + g * 384: k * 768 + g * 384 + 384],
                            start=(b == 0), stop=(b == 7))
                st = tmp.tile([128, 384], F32, name=f"st{t}", tag="st")
                nc.vector.tensor_add(st[:], fcp[:], cf[:])
                nc.sync.dma_start(out_d[t * 128:(t + 1) * 128, :], st[:])

            def emit_chain_head(t, pr, phn):
                # r = Sig(pre_r); t1 = r * phn  (column-chunked to pipeline)
                r = tmp.tile([128, 512], F32, name=f"r{t}", tag="r")
                t1 = tmp.tile([128, 512], F32, name=f"t1{t}", tag="t1")
                for a in range(2):
                    s = slice(a * 256, (a + 1) * 256)
                    nc.scalar.activation(r[:, s], pr(a), Sig)
                    nc.vector.tensor_tensor(t1[:, s], r[:, s], phn(a), MUL)
                return t1

            def emit_chain_mid(t, t1, pin):
                # t2 = t1 + i_n; n = Tanh(t2)  (column-chunked to pipeline)
                t2 = tmp.tile([128, 512], F32, name=f"t2{t}", tag="t2")
                n = tmp.tile([128, 512], F16, name=f"n{t}", tag="n")
                for a in range(2):
                    s = slice(a * 256, (a + 1) * 256)
                    nc.vector.tensor_tensor(t2[:, s], t1[:, s], pin(a), ADD)
                    nc.scalar.activation(n[:, s], t2[:, s], Tanh)
                return n

            def emit_chain_tail(t, n, pz):
                # d = h - n; per 256-col chunk: z=Sig(pre_z), e=z*d, h'=n+e
                z = tmp.tile([128, 512], F16, name=f"z{t}", tag="z")
                d = tmp.tile([128, 512], F16, name=f"d{t}", tag="d")
                e = tmp.tile([128, 512], F16, name=f"e{t}", tag="e")
                for a in range(2):
                    s = slice(a * 256, (a + 1) * 256)
                    nc.vector.tensor_tensor(d[:, s], h[:, s], n[:, s], SUB)
                for a in range(2):
                    s = slice(a * 256, (a + 1) * 256)
                    nc.scalar.activation(z[:, s], pz(a), Sig)
                    nc.vector.tensor_tensor(e[:, s], z[:, s], d[:, s], MUL)
                    nc.vector.tensor_tensor(h[:, s], n[:, s], e[:, s], ADD)

            def emit_inits():
                for j in (3, 0, 2):
                    emit_init_bank(j)
                emit_init_z(0)
                emit_init_z(1)

            # ---- step 0: gates come from host-computed G0 ----
            t1_0 = emit_chain_head(0,
                                   lambda a: g0[:, a * 256:(a + 1) * 256],
                                   lambda a: g0[:, 1536 + a * 256: 1536 + (a + 1) * 256])
            n_0 = emit_chain_mid(0, t1_0,
                                 lambda a: g0[:, 1024 + a * 256: 1024 + (a + 1) * 256])
            emit_chain_tail(0, n_0,
                            lambda a: g0[:, 512 + a * 256: 512 + (a + 1) * 256])
            # Step 0: transposes+fc first — the bias inits wait on the CINIT
            # DMA (last small input) and would serialize the prologue if they
            # sat ahead in the PE queue; their only deadline is step 1's G.
            emit_transposes_fc(0)
            if T > 1:
                emit_inits()

            for t in range(1, T):
                emit_G_bank(3)   # h_n
                emit_G_bank(0)   # r
                t1_t = emit_chain_head(t,
                                       lambda a: gb[0][:, a * 256:(a + 1) * 256],
                                       lambda a: gb[3][:, a * 256:(a + 1) * 256])
                emit_G_bank(2)   # i_n
                n_t = emit_chain_mid(t, t1_t,
                                     lambda a: gb[2][:, a * 256:(a + 1) * 256])
                emit_G_z(0)      # z first half  (tail starts on its result)
                emit_G_z(1)      # z second half
                emit_chain_tail(t, n_t, lambda a: gz[a][:, :])
                if t + 1 < T:
                    emit_inits()
                emit_transposes_fc(t)

    nc.compile()
    return nc


def _hi_lo(x):
    hi = x.astype(np.float16)
    lo = (x - hi.astype(np.float32)).astype(np.float16)
    return hi, lo


def _split_cols(x):
    """[B, 1024] -> [128, 512] split layout (dims 0-511 on parts 0-63)."""
    return np.concatenate([x[:, :512], x[:, 512:]], axis=0)


def kernel(src, hidden, W_ih, W_hh, b_ih, b_hh, fc_W, fc_b, output_len):
    from concourse import bass_utils

    T = int(output_len)
    src = np.asarray(src, np.float32)
    hidden = np.asarray(hidden, np.float32)
    W_ih = np.asarray(W_ih, np.float32)
    W_hh = np.asarray(W_hh, np.float32)
    b_ih = np.asarray(b_ih, np.float32)
    b_hh = np.asarray(b_hh, np.float32)
    fc_W = np.asarray(fc_W, np.float32)
    fc_b = np.asarray(fc_b, np.float32)

    ctx = hidden[0]          # [B, H]
    h0 = hidden[0]
    x0 = src[0]              # [B, OUT]

    # ---- host weight folding (fp32) ----
    M1 = W_ih @ fc_W[:, :H]          # [3H, H]
    M2 = W_ih @ fc_W[:, H:]          # [3H, H]
    P_r = M1[0:H] + W_hh[0:H]
    P_z = M1[H:2 * H] + W_hh[H:2 * H]
    Wbig4 = np.concatenate([P_r, P_z, M1[2 * H:], W_hh[2 * H:]], axis=0)  # [4096, H]
    F1 = fc_W[:, :H]                 # [OUT, H]

    CONST = ctx @ M2.T + (fc_b @ W_ih.T + b_ih)     # [B, 3H]
    c_r = CONST[:, 0:H] + b_hh[0:H]
    c_z = CONST[:, H:2 * H] + b_hh[H:2 * H]
    c_in = CONST[:, 2 * H:]
    c_hn = np.broadcast_to(b_hh[2 * H:], (BATCH, H)).astype(np.float32)
    CALL = np.stack([c_r, c_z, c_in, c_hn], axis=1)  # [B, 4, H]

    CF = ctx @ fc_W[:, H:].T + fc_b                  # [B, OUT]

    gi0 = x0 @ W_ih.T + b_ih
    gh0 = h0 @ W_hh.T + b_hh
    G0_parts = np.stack([gi0[:, :H] + gh0[:, :H],
                         gi0[:, H:2 * H] + gh0[:, H:2 * H],
                         gi0[:, 2 * H:],
                         gh0[:, 2 * H:]], axis=1)    # [B, 4, H]

    # ---- shared (replicated) tensors ----
    # W4 sbuf layout: [p, k*4096 + cc*512 + c] = Wbig4[1024*j + 512*g + c, 128k+p]
    W4T = Wbig4.T.reshape(8, 128, 8, 512)            # [k, p, cc, c]
    W4 = np.ascontiguousarray(W4T.transpose(1, 0, 2, 3)).reshape(128, 8 * 4096)
    W4 = W4.astype(np.float16)
    # F1 sbuf: [p, k*768 + g*384 + c] = F1[384g+c, 128k+p]
    F1T = F1.T.reshape(8, 128, 2, 384)               # [k, p, g, c]
    F1s = np.ascontiguousarray(F1T.transpose(1, 0, 2, 3)).reshape(128, 8 * 768)
    F1s = F1s.astype(np.float16)
    ID2 = np.concatenate([np.eye(64), np.eye(64)], axis=0).astype(np.float16)
    IDT = np.eye(128).astype(np.float16)

    key = T
    if key not in _BUILD_CACHE:
        _BUILD_CACHE[key] = _build(T)
    nc = _BUILD_CACHE[key]

    in_maps = []
    for c in range(NCORES):
        sl = slice(c * B, (c + 1) * B)
        # CINIT: [p, (j*2+g)*512 + c]: p<64 hi, p>=64 lo of CALL[b, j, 512g+c]
        call_c = CALL[sl].reshape(B, 4, 2, 512)      # [b, j, g, c]
        hi, lo = _hi_lo(call_c)
        ci = np.concatenate([hi, lo], axis=0)        # [128, 4, 2, 512]
        ci = np.ascontiguousarray(ci).reshape(128, 8 * 512)

        h0_c = h0[sl]
        H0S = _split_cols(h0_c)                      # [128, 512]

        g0_c = G0_parts[sl]                          # [B, 4, H]
        G0s = np.concatenate([g0_c[:, :, :512], g0_c[:, :, 512:]],
                             axis=0)                 # [128, 4, 512]
        G0s = np.ascontiguousarray(G0s).reshape(128, 4 * 512)

        cf_c = CF[sl].reshape(B, 2, 384)             # [b, g, c]
        CFs = np.ascontiguousarray(cf_c.transpose(1, 0, 2)).reshape(128, 384)

        in_maps.append({
            "W4": W4, "F1": F1s,
            "CINIT": np.ascontiguousarray(ci).astype(np.float16),
            "IDENT2": ID2, "IDENTT": IDT,
            "H0S": np.ascontiguousarray(H0S).astype(np.float16),
            "G0": G0s.astype(np.float32),
            "CF": CFs.astype(np.float32),
        })

    trace = bool(os.environ.get("GRU_TRACE"))
    res = bass_utils.run_bass_kernel_spmd(
        nc, in_maps, core_ids=list(range(NCORES)), trace=trace)
    if trace:
        kernel.last_exec_time_ns = res.exec_time_ns
        kernel.last_results = res

    outs = []
    for c in range(NCORES):
        o = res.results[c]["OUT"]                    # [T*128, 384]
        o = o.reshape(T, 2, B, 384).transpose(0, 2, 1, 3).reshape(T, B, OUT)
        outs.append(o)
    return np.concatenate(outs, axis=1)              # [T, BATCH, OUT]
